# revision 8
# baseline (speedup 1.0000x reference)
"""BiCrossAttention Trainium2 kernel.

Shards the (B=2, H=8) problem across 8 NeuronCores as (batch, head-pair):
core c handles batch c//4 and heads {2*(c%4), 2*(c%4)+1}.  Each core
computes its two heads' QKV projections, both cross-attention branches,
and a partial output projection; the host sums the 4 per-batch partials
and adds the bias.

Device-side layout notes:
  - activations are passed pre-transposed/tiled: xT[p, kc, n] = x[n, kc*128+p]
  - matmuls run in bf16 (1 cyc/row); scores are computed transposed
    (simT[j, i]) so exp feeds the attn@V matmul directly as a stationary
    operand
  - the inner loop processes (branch0, head X) and (branch1, head Y)
    together: their K=64 score matmuls occupy disjoint PE row groups
    (partitions 0-63 vs 64-127) and run concurrently, and one Exp
    instruction covers both members' scores
  - V is transposed via the DMA xbar (dma_start(transpose=True)), not the
    PE; the two heads share a single softmax-denominator ones column:
    vaug layout is [v_h0 (64) | ones@64 | pad | ones@79 | v_h1 @80..143],
    so head 0's stationary is cols 0:65 (denominator lands on acc row 64)
    and head 1's is cols 79:144 (denominator on acc row 0, values on rows
    1:65)
  - the softmax normalization is split: the acc-PSUM reads (value copy,
    reciprocal, f32r cast) are emitted inline at the slab boundary on DVE
    only, while the PE work (reciprocal broadcast matmul, mul/add, DMA
    head-shift, output projection) is deferred into the next slab's inner
    loop as pumped filler, keeping the PE stream dense so ACT (the exp
    engine, the true bottleneck at ~1.1us per iteration) never starves
  - QKV projection chunks are interleaved into the attention loop as PE
    filler; the kv/vaug streaming in the first slab pass is paced one
    piece per j-chunk
  - alpha gating is folded into the V weights on the host
"""

import sys
import types

import numpy as np

for _p in ("/opt/trn_rl_repo",):
    if _p not in sys.path:
        sys.path.append(_p)

# Register the axon NTFF profile hook if the image's antenv lacks it (needed
# only when tracing; harmless otherwise).
try:
    import antenv

    if "antenv.axon_hooks" not in sys.modules:
        try:
            import antenv.axon_hooks  # noqa: F401
        except ImportError:
            _hooks = types.ModuleType("antenv.axon_hooks")
            _hook_holder = [None]
            _hooks.set_axon_ntff_profile_hook = lambda h: _hook_holder.__setitem__(0, h)
            _hooks.get_axon_ntff_profile_hook = lambda: _hook_holder[0]
            sys.modules["antenv.axon_hooks"] = _hooks
            antenv.axon_hooks = _hooks
            try:
                from trn_agent_boot.trn_boot import _ntff_profile_via_ctypes

                _hooks.set_axon_ntff_profile_hook(
                    _ntff_profile_via_ctypes("/opt/axon/libaxon_pjrt.so")
                )
            except Exception:
                pass
except Exception:
    pass

import ml_dtypes
import concourse.bacc as bacc
import concourse.mybir as mybir
import concourse.tile as tile
from concourse import bass_utils

F32 = mybir.dt.float32
F32R = mybir.dt.float32r
BF16 = mybir.dt.bfloat16

_NP = {F32: np.float32, F32R: np.float32, BF16: ml_dtypes.bfloat16}

# Full problem constants
B, N, QD, CD, H, DH = 2, 2048, 1024, 1024, 8, 64
INNER = H * DH
SCALE = DH**-0.5
N_CORES = 8
HG = 4  # head-groups (of 2 heads) per batch

# vaug layout columns
V0C = 0     # head-0 values, cols 0:64
ONE0 = 64   # ones column for head 0 (stationary cols 0:65)
ONE1 = 79   # ones column for head 1 (stationary cols 79:144)
V1C = 80    # head-1 values, cols 80:144 (32B-aligned for the DMA xbar)
VW = 144


class Cfg:
    def __init__(self, n=N, d=QD, dt_proj=BF16, dt_attn=BF16, dt_out=BF16):
        self.N = n          # sequence length
        self.D = d          # model dim (= QD = CD)
        self.KC = d // 128  # contraction chunks for projections
        self.ISLAB = min(512, n)   # attention i-slab / projection i-chunk
        self.NJC = n // 128  # j chunks (128 keys each)
        self.dt_proj = dt_proj
        self.dt_attn = dt_attn
        self.dt_out = dt_out


def build_nc(cfg: Cfg):
    """Builds the single-core program (SPMD across all 8 cores)."""
    nc = bacc.Bacc("TRN2", target_bir_lowering=False, debug=False)
    KC, Nn, D = cfg.KC, cfg.N, cfg.D
    ISLAB, NJC = cfg.ISLAB, cfg.NJC
    NSL = Nn // ISLAB
    NCH = Nn // ISLAB  # projection chunks per tensor
    DTP, DTA, DTO = cfg.dt_proj, cfg.dt_attn, cfg.dt_out

    NCH_ = Nn // min(512, Nn)
    xT = nc.dram_tensor(
        "xT", [NCH_, 128, KC, min(512, Nn)], DTP, kind="ExternalInput"
    ).ap()
    cT = nc.dram_tensor(
        "cT", [NCH_, 128, KC, min(512, Nn)], DTP, kind="ExternalInput"
    ).ap()
    wd = {
        name: nc.dram_tensor(name, [128, KC, 128], DTP, kind="ExternalInput").ap()
        for name in ("wq1", "wk1", "wv1", "wq2", "wk2", "wv2")
    }
    wout_d = nc.dram_tensor("wout", [128, D], DTO, kind="ExternalInput").ap()
    y_d = nc.dram_tensor("y", [Nn, D], F32, kind="ExternalOutput").ap()

    with tile.TileContext(nc) as tc:
        with (
            tc.tile_pool(name="const", bufs=1) as cpool,
            tc.tile_pool(name="qkv", bufs=1) as qkvpool,
            tc.tile_pool(name="vaug", bufs=1) as vaugpool,
            tc.tile_pool(name="outp", bufs=1) as outpool,
            tc.tile_pool(name="slab", bufs=8) as slabpool,
            tc.tile_pool(name="exp", bufs=6) as exppool,
            tc.tile_pool(name="tmp", bufs=4) as tmppool,
            tc.tile_pool(name="ysb", bufs=3) as ypool,
            tc.tile_pool(name="sim", bufs=2, space="PSUM") as simpool,
            tc.tile_pool(name="acc", bufs=2, space="PSUM") as accpool,
            tc.tile_pool(name="util", bufs=2, space="PSUM") as utilpool,
        ):
            # ---- input/weight DMAs, most-urgent first ----
            # chunk-0 activations and the q2/k1 weights gate the first score
            # matmul; everything else streams behind them.
            xs_tiles = {}
            SRCD = {"1": xT, "2": cT}

            def queue_src_slab(stream, ch):
                xs = slabpool.tile(
                    [128, KC, ISLAB], DTP, tag="xs", name=f"xs{stream}{ch}"
                )
                nc.sync.dma_start(out=xs[:], in_=SRCD[stream][ch])
                xs_tiles[(stream, ch)] = xs

            queue_src_slab("2", 0)
            queue_src_slab("1", 0)
            w_sb = {}
            for name in ("wq2", "wk1", "wq1", "wk2", "wv1", "wv2"):
                w_sb[name] = cpool.tile([128, KC, 128], DTP, tag=name, name=name)
                nc.sync.dma_start(out=w_sb[name][:], in_=wd[name])
            for ch in range(1, NCH):
                queue_src_slab("2", ch)
                queue_src_slab("1", ch)
            wout_sb = cpool.tile([128, D], DTO, tag="wout")
            nc.sync.dma_start(out=wout_sb[:], in_=wout_d)

            ones_f32 = cpool.tile([128, 65], F32, tag="ones_f32")
            nc.vector.memset(ones_f32[:], 1.0)
            ones_sb = cpool.tile([128, 65], F32R, tag="ones")
            nc.vector.tensor_copy(out=ones_sb[:], in_=ones_f32[:])

            proj = {}
            for name in ("q1", "k1", "v1", "q2", "k2", "v2"):
                proj[name] = qkvpool.tile([128, Nn], DTA, tag=name, name=name)

            # vaug[br][j, jc, c]: per-branch transposed V for both heads with
            # shared ones columns (layout in module docstring).
            vaug = {}
            for br in (0, 1):
                vaug[br] = vaugpool.tile(
                    [128, NJC, VW], DTA, tag=f"vaug{br}", name=f"vaug{br}"
                )
                for onec in (ONE0, ONE1):
                    nc.vector.tensor_copy(
                        out=vaug[br][:, :, onec],
                        in_=ones_f32[:, 0:1].to_broadcast((128, NJC)),
                    )

            # outT rows 0..63 = head 0 (written directly); head 1 staged in
            # outTB rows 1..64 then DMA-shifted into rows 64..127 (compute
            # engines are lane-aligned; DMA can move across partitions).
            outT = outpool.tile([128, Nn], DTO, tag="outT")
            outTB = outpool.tile([65, Nn], DTO, tag="outTB")

            # ---- streamed projection chunks (PE filler inside attention) ----
            chunk_done = set()
            chunk_done.add(("src", "1", 0))
            chunk_done.add(("src", "2", 0))
            for ch in range(1, NCH):
                chunk_done.add(("src", "1", ch))
                chunk_done.add(("src", "2", ch))

            def ensure_chunk(pname, ch):
                """Project chunk ch (ISLAB wide) of tensor pname."""
                if (pname, ch) in chunk_done or ch >= NCH:
                    return
                chunk_done.add((pname, ch))
                stream = pname[1]
                xs = xs_tiles[(stream, ch)]
                pp = utilpool.tile([128, ISLAB], F32, tag="util", name=f"pp{pname}{ch}")
                wt = w_sb["w" + pname]
                for kc in range(KC):
                    nc.tensor.matmul(
                        pp[:], wt[:, kc, :], xs[:, kc, :],
                        start=(kc == 0), stop=(kc == KC - 1),
                    )
                nc.vector.tensor_copy(
                    out=proj[pname][:, ch * ISLAB : (ch + 1) * ISLAB], in_=pp[:]
                )

            def ensure_vaug(br, ch):
                """Transpose chunk ch of v{br+1} into vaug via the DMA xbar."""
                if ("vaug", br, ch) in chunk_done or ch >= NCH:
                    return
                chunk_done.add(("vaug", br, ch))
                vname = "v1" if br == 0 else "v2"
                ensure_chunk(vname, ch)
                vt = proj[vname]
                for jc in range(ch * (ISLAB // 128), (ch + 1) * (ISLAB // 128)):
                    jsl = slice(jc * 128, (jc + 1) * 128)
                    nc.sync.dma_start(
                        out=vaug[br][:, jc, V0C : V0C + 64],
                        in_=vt[0:64, jsl], transpose=True,
                    )
                    nc.sync.dma_start(
                        out=vaug[br][:, jc, V1C : V1C + 64],
                        in_=vt[64:128, jsl], transpose=True,
                    )

            # Deferred PE work (output projection chunks, late q-projection
            # chunks, slab-normalize PE pieces) is queued and pumped a couple
            # of jobs per j-chunk so it fills PE slack without delaying the
            # score matmuls that feed ACT.
            jobs = []

            def pump(n=1):
                for _ in range(min(n, len(jobs))):
                    jobs.pop(0)()

            def queue_qchunk(pname, ch):
                """Front-queue one q-projection chunk as two half jobs."""
                if (pname, ch) in chunk_done or ch >= NCH:
                    return
                chunk_done.add((pname, ch))
                stream = pname[1]
                holder = {}

                def half1():
                    xs = xs_tiles[(stream, ch)]
                    pp = utilpool.tile(
                        [128, ISLAB], F32, tag="util", name=f"pp{pname}{ch}"
                    )
                    holder["pp"] = pp
                    wt = w_sb["w" + pname]
                    for kc in range(KC // 2):
                        nc.tensor.matmul(
                            pp[:], wt[:, kc, :], xs[:, kc, :],
                            start=(kc == 0), stop=False,
                        )

                def half2():
                    xs = xs_tiles[(stream, ch)]
                    pp = holder["pp"]
                    wt = w_sb["w" + pname]
                    for kc in range(KC // 2, KC):
                        nc.tensor.matmul(
                            pp[:], wt[:, kc, :], xs[:, kc, :],
                            start=False, stop=(kc == KC - 1),
                        )
                    nc.vector.tensor_copy(
                        out=proj[pname][:, ch * ISLAB : (ch + 1) * ISLAB],
                        in_=pp[:],
                    )

                jobs[:0] = [half1, half2]

            def queue_outproj(sl, last=False):
                ocw = min(512, D)
                nocs = D // ocw
                copy_eng = nc.scalar if last else nc.vector
                out = []
                for ic in range(sl * (ISLAB // 128), (sl + 1) * (ISLAB // 128)):
                    ysb = ypool.tile([128, D], F32, tag="ysb", name=f"ysb{ic}")

                    def job(ic=ic, ysb=ysb, oc=0):
                        icsl = slice(ic * 128, (ic + 1) * 128)
                        ocsl = slice(oc * ocw, (oc + 1) * ocw)
                        py = utilpool.tile(
                            [128, ISLAB], F32, tag="util", name=f"py{ic}{oc}"
                        )
                        nc.tensor.matmul(
                            py[:, 0:ocw], outT[:, icsl], wout_sb[:, ocsl],
                            start=True, stop=True,
                        )
                        if last:
                            nc.scalar.copy(out=ysb[:, ocsl], in_=py[:, 0:ocw])
                        else:
                            nc.vector.tensor_copy(
                                out=ysb[:, ocsl], in_=py[:, 0:ocw]
                            )
                        nc.sync.dma_start(
                            out=y_d[ic * 128 : (ic + 1) * 128, ocsl],
                            in_=ysb[:, ocsl],
                        )

                    for oc in range(nocs):
                        out.append(lambda ic=ic, ysb=ysb, oc=oc: job(ic, ysb, oc))
                return out

            # pair p=0: (br0, head0) + (br1, head1); p=1: (br0, head1) + (br1, head0)
            PAIRS = (((0, 0), (1, 1)), ((0, 1), (1, 0)))
            JPC = ISLAB // 128  # j-chunks per projection chunk

            def emit_scores(members, simP, sl, jc):
                i0 = sl * ISLAB
                jsl = slice(jc * 128, (jc + 1) * 128)
                for m, (br, h) in enumerate(members):
                    q = proj["q2"] if br == 0 else proj["q1"]
                    k = proj["k1"] if br == 0 else proj["k2"]
                    rs = slice(h * 64, h * 64 + 64)
                    nc.tensor.matmul(
                        simP[:, m, :], k[rs, jsl], q[rs, i0 : i0 + ISLAB],
                        start=True, stop=True, tile_position=(h * 64, 0),
                    )

            def emit_attnv(members, expP, accs, jc):
                for m, (br, h) in enumerate(members):
                    vcols = slice(V0C, V0C + 65) if h == 0 else slice(ONE1, VW)
                    nc.tensor.matmul(
                        accs[(br, h)][0:65, :], vaug[br][:, jc, vcols],
                        expP[:, m, :],
                        start=(jc == 0), stop=(jc == NJC - 1),
                    )

            def finish_slab(p, sl, accs, last=False):
                """Normalize both members of a finished slab.

                Emits the acc-PSUM reads (DVE only) inline so the acc slots
                rotate; returns the PE/mixed closures (broadcast matmul,
                mul/add, head-1 shift, output projection) for deferred
                pumping -- unless last, in which case everything is emitted
                now.
                """
                members = PAIRS[p]
                i0 = sl * ISLAB
                isl_ = slice(i0, i0 + ISLAB)
                deferred = []
                for br, h in members:
                    acc = accs[(br, h)]
                    # h1's denominator sits on row 0 with values on rows 1:65;
                    # PSUM partition bases must be 32-aligned, so h1 ops cover
                    # rows 0:65 and row 0 harmlessly computes den * (1/den)
                    vr = slice(0, 64) if h == 0 else slice(0, 65)
                    drow = 64 if h == 0 else 0
                    ot = outT if h == 0 else outTB
                    orows = slice(0, 64) if h == 0 else slice(0, 65)
                    # inline: drain acc (copy + reciprocal) and stage 1/den
                    if p == 0:
                        dst = ot[orows, isl_]
                    else:
                        tmp = tmppool.tile(
                            [128, ISLAB], F32, tag="tmp", name=f"tmp{sl}{br}{h}"
                        )
                        dst = tmp[vr, :]
                    nc.vector.tensor_copy(out=dst, in_=acc[vr, :])
                    rcpf = tmppool.tile(
                        [128, ISLAB], F32, tag="rcpf", name=f"rcpf{sl}{br}{h}"
                    )
                    # the custom-DVE reciprocal requires base partition 0
                    nc.vector.reciprocal_approx_fast(
                        out=rcpf[0 : drow + 1, :], in_=acc[0 : drow + 1, :]
                    )
                    rcp = tmppool.tile(
                        [128, ISLAB], F32R, tag="rcpr", name=f"rcp{sl}{br}{h}"
                    )
                    nc.vector.tensor_copy(
                        out=rcp[drow : drow + 1, :], in_=rcpf[drow : drow + 1, :]
                    )

                    def c_bc(br=br, h=h, drow=drow, rcp=rcp):
                        bc = utilpool.tile(
                            [128, ISLAB], F32, tag="util", name=f"bc{sl}{br}{h}"
                        )
                        # h0: den on row 64, broadcast to rows 0:64 (M=64)
                        # h1: den on row 0, broadcast to rows 0:65 (M=65)
                        mcols = 64 if drow == 64 else 65
                        nc.tensor.matmul(
                            bc[0:mcols, :],
                            ones_sb[drow : drow + 1, 0:mcols],
                            rcp[drow : drow + 1, :],
                            start=True, stop=True,
                        )
                        bc_holder[(br, h)] = bc

                    def c_mul(br=br, h=h, vr=vr, ot=ot, orows=orows, dst=dst):
                        bc = bc_holder[(br, h)]
                        nc.vector.tensor_mul(out=dst, in0=dst, in1=bc[vr, :])
                        if p != 0:
                            nc.vector.tensor_add(
                                out=ot[orows, isl_], in0=ot[orows, isl_], in1=dst
                            )

                    deferred.append(c_bc)
                    deferred.append(c_mul)
                if p == 1:
                    def c_shift():
                        nc.sync.dma_start(
                            out=outT[64:128, isl_], in_=outTB[1:65, isl_]
                        )

                    deferred.append(c_shift)
                    deferred.extend(queue_outproj(sl, last=last))
                if last:
                    for c in deferred:
                        c()
                else:
                    jobs[:0] = deferred

            bc_holder = {}

            # ---- first iteration, unrolled for fastest ACT spin-up ----
            members0 = PAIRS[0]
            ensure_chunk("q2", 0)
            ensure_chunk("k1", 0)
            accs = {
                (br, h): accpool.tile([128, ISLAB], F32, tag="acc", name=f"acc{br}{h}")
                for br, h in members0
            }
            simP = simpool.tile([128, 2, ISLAB], F32, tag="sim")
            expP = exppool.tile([128, 2, ISLAB], DTA, tag="exp")
            emit_scores(members0[:1], simP, 0, 0)
            nc.scalar.activation(
                expP[:, 0, :], simP[:, 0, :],
                mybir.ActivationFunctionType.Exp, scale=SCALE,
            )
            ensure_chunk("q1", 0)
            ensure_chunk("k2", 0)
            i0 = 0
            jsl = slice(0, 128)
            br, h = members0[1]
            q = proj["q2"] if br == 0 else proj["q1"]
            k = proj["k1"] if br == 0 else proj["k2"]
            rs = slice(h * 64, h * 64 + 64)
            nc.tensor.matmul(
                simP[:, 1, :], k[rs, jsl], q[rs, 0:ISLAB],
                start=True, stop=True, tile_position=(h * 64, 0),
            )
            nc.scalar.activation(
                expP[:, 1, :], simP[:, 1, :],
                mybir.ActivationFunctionType.Exp, scale=SCALE,
            )
            ensure_vaug(0, 0)
            ensure_vaug(1, 0)
            emit_attnv(members0, expP, accs, 0)

            for p, members in enumerate(PAIRS):
                for sl in range(NSL):
                    first = p == 0 and sl == 0
                    if not first:
                        accs = {
                            (br, h): accpool.tile(
                                [128, ISLAB], F32, tag="acc", name=f"acc{br}{h}"
                            )
                            for br, h in members
                        }
                    for jc in range(1 if first else 0, NJC):
                        if first:
                            # paced k/v/vaug streaming, one piece per j-chunk;
                            # chunk c's four pieces land on jc = 4(c-1)+1 .. 4c,
                            # just before its own j-chunks begin at jc = 4c
                            nxt = (jc - 1) // JPC + 1
                            if nxt < NCH:
                                ph = (jc - 1) % JPC
                                if ph == 0:
                                    ensure_chunk("k2", nxt)
                                elif ph == 1:
                                    ensure_chunk("k1", nxt)
                                elif ph == 2:
                                    ensure_vaug(0, nxt)
                                else:
                                    ensure_vaug(1, nxt)
                        if p == 0 and jc in (8, 10) and sl + 1 < NSL:
                            queue_qchunk("q2" if jc == 8 else "q1", sl + 1)

                        simP = simpool.tile([128, 2, ISLAB], F32, tag="sim")
                        emit_scores(members, simP, sl, jc)
                        expP = exppool.tile([128, 2, ISLAB], DTA, tag="exp")
                        nc.scalar.activation(
                            expP[:], simP[:],
                            mybir.ActivationFunctionType.Exp, scale=SCALE,
                        )
                        # deferred work lands between exp and attn@V in the PE
                        # stream, where the PE is waiting on ACT anyway
                        pump(2)
                        emit_attnv(members, expP, accs, jc)

                    finish_slab(p, sl, accs, last=(p == 1 and sl == NSL - 1))
            pump(len(jobs))

    nc.compile()
    return nc


_CACHE = {}
_ACTIVE_CFG = Cfg()


def _get_nc():
    if "nc" not in _CACHE:
        _CACHE["nc"] = build_nc(_ACTIVE_CFG)
    return _CACHE["nc"]


def _tile_kpart(a, dt):
    """[K, M] -> [128, K//128, M] with element (p, kc, m) = a[kc*128+p, m]."""
    k, m = a.shape
    return np.ascontiguousarray(
        a.reshape(k // 128, 128, m).transpose(1, 0, 2)
    ).astype(_NP[dt])


def make_in_maps(x, context, Wq1, Wk1, Wv1, Wq2, Wk2, Wv2, alpha_attn, Wout, bout):
    cfg = _ACTIVE_CFG
    alpha = float(1.0 / (1.0 + np.exp(-np.float64(alpha_attn))))
    Wv1s = np.asarray(Wv1, np.float32) * np.float32(alpha)
    Wv2s = np.asarray(Wv2, np.float32) * np.float32(1.0 - alpha)

    def _chunked(a):
        t = _tile_kpart(a, cfg.dt_proj)  # [128, KC, N]
        w = min(512, cfg.N)
        return np.ascontiguousarray(
            t.reshape(128, cfg.KC, cfg.N // w, w).transpose(2, 0, 1, 3)
        )

    xT = [_chunked(np.asarray(x[b], np.float32).T) for b in range(B)]
    cT = [_chunked(np.asarray(context[b], np.float32).T) for b in range(B)]

    in_maps = []
    for c in range(N_CORES):
        b, hg = c // HG, c % HG
        cols = slice(hg * 128, (hg + 1) * 128)
        in_maps.append(
            {
                "xT": xT[b],
                "cT": cT[b],
                "wq1": _tile_kpart(np.asarray(Wq1, np.float32)[:, cols], cfg.dt_proj),
                "wk1": _tile_kpart(np.asarray(Wk1, np.float32)[:, cols], cfg.dt_proj),
                "wv1": _tile_kpart(Wv1s[:, cols], cfg.dt_proj),
                "wq2": _tile_kpart(np.asarray(Wq2, np.float32)[:, cols], cfg.dt_proj),
                "wk2": _tile_kpart(np.asarray(Wk2, np.float32)[:, cols], cfg.dt_proj),
                "wv2": _tile_kpart(Wv2s[:, cols], cfg.dt_proj),
                "wout": np.ascontiguousarray(
                    np.asarray(Wout, np.float32)[cols, :]
                ).astype(_NP[cfg.dt_out]),
            }
        )
    return in_maps


def run_device(in_maps, trace=False, tmpdir=None):
    nc = _get_nc()
    return bass_utils.run_bass_kernel_spmd(
        nc, in_maps, core_ids=list(range(N_CORES)), trace=trace, tmpdir=tmpdir
    )


def kernel(x, context, Wq1, Wk1, Wv1, Wq2, Wk2, Wv2, alpha_attn, Wout, bout):
    in_maps = make_in_maps(
        x, context, Wq1, Wk1, Wv1, Wq2, Wk2, Wv2, alpha_attn, Wout, bout
    )
    res = run_device(in_maps)
    bout32 = np.asarray(bout, np.float32)
    out = np.empty((B, N, QD), np.float32)
    for b in range(B):
        acc = res.results[b * HG]["y"].astype(np.float32).copy()
        for hg in range(1, HG):
            acc += res.results[b * HG + hg]["y"]
        out[b] = acc + bout32[None, :]
    return out


# revision 19
# speedup vs baseline: 1.2321x; 1.2321x over previous
"""BiCrossAttention Trainium2 kernel.

Shards the (B=2, H=8) problem across 8 NeuronCores as (batch, head-pair):
core c handles batch c//4 and heads {2*(c%4), 2*(c%4)+1}.  Each core
computes its two heads' QKV projections, both cross-attention branches,
and a partial output projection; the host sums the 4 per-batch partials
and adds the bias.

Device-side layout notes:
  - activations are passed pre-transposed/tiled: xT[p, kc, n] = x[n, kc*128+p]
  - matmuls run in bf16 (1 cyc/row); scores are computed transposed
    (simT[j, i]) so exp feeds the attn@V matmul directly as a stationary
    operand
  - the inner loop processes (branch0, head X) and (branch1, head Y)
    together: their K=64 score matmuls occupy disjoint PE row groups
    (partitions 0-63 vs 64-127) and run concurrently, and one Exp
    instruction covers both members' scores
  - V is projected wide (like q/k) then transposed per 128-j-chunk on the
    PE; a single strided cast drops the transposed [j, 2*64] block into
    vaug[br][j, jc, head, 0:64], with a softmax-denominator ones column
    at [.., head, 64] (so both heads' denominators land on acc row 64)
  - the softmax normalization is split: the acc-PSUM reads (value copy,
    reciprocal, f32r cast) are emitted inline at the slab boundary on DVE
    only, while the PE work (reciprocal broadcast matmul, mul/add, DMA
    head-shift, output projection) is deferred into the next slab's inner
    loop as pumped filler, keeping the PE stream dense so ACT (the exp
    engine, the true bottleneck at ~1.1us per iteration) never starves
  - QKV projection chunks are interleaved into the attention loop as PE
    filler; kv/vT streaming in the first slab pass is paced per j-chunk
  - alpha gating is folded into the V weights on the host
"""

import sys
import types

import numpy as np

for _p in ("/opt/trn_rl_repo",):
    if _p not in sys.path:
        sys.path.append(_p)

# Register the axon NTFF profile hook if the image's antenv lacks it (needed
# only when tracing; harmless otherwise).
try:
    import antenv

    if "antenv.axon_hooks" not in sys.modules:
        try:
            import antenv.axon_hooks  # noqa: F401
        except ImportError:
            _hooks = types.ModuleType("antenv.axon_hooks")
            _hook_holder = [None]
            _hooks.set_axon_ntff_profile_hook = lambda h: _hook_holder.__setitem__(0, h)
            _hooks.get_axon_ntff_profile_hook = lambda: _hook_holder[0]
            sys.modules["antenv.axon_hooks"] = _hooks
            antenv.axon_hooks = _hooks
            try:
                from trn_agent_boot.trn_boot import _ntff_profile_via_ctypes

                _hooks.set_axon_ntff_profile_hook(
                    _ntff_profile_via_ctypes("/opt/axon/libaxon_pjrt.so")
                )
            except Exception:
                pass
except Exception:
    pass

import ml_dtypes
import concourse.bacc as bacc
import concourse.mybir as mybir
import concourse.tile as tile
from concourse import bass_utils
from concourse.masks import make_identity

F32 = mybir.dt.float32
F32R = mybir.dt.float32r
BF16 = mybir.dt.bfloat16

_NP = {F32: np.float32, F32R: np.float32, BF16: ml_dtypes.bfloat16}

# Full problem constants
B, N, QD, CD, H, DH = 2, 2048, 1024, 1024, 8, 64
INNER = H * DH
SCALE = DH**-0.5
N_CORES = 8
HG = 4  # head-groups (of 2 heads) per batch

# vaug layout: [128 j, NJC, 2 heads, VHW] with [v (64) | ones | pad]
VHW = 80


class Cfg:
    def __init__(self, n=N, d=QD, dt_proj=BF16, dt_attn=BF16, dt_out=BF16):
        self.N = n          # sequence length
        self.D = d          # model dim (= QD = CD)
        self.KC = d // 128  # contraction chunks for projections
        self.ISLAB = min(512, n)   # attention i-slab / projection i-chunk
        self.NJC = n // 128  # j chunks (128 keys each)
        self.dt_proj = dt_proj
        self.dt_attn = dt_attn
        self.dt_out = dt_out


def build_nc(cfg: Cfg):
    """Builds the single-core program (SPMD across all 8 cores)."""
    nc = bacc.Bacc("TRN2", target_bir_lowering=False, debug=False)
    KC, Nn, D = cfg.KC, cfg.N, cfg.D
    ISLAB, NJC = cfg.ISLAB, cfg.NJC
    NSL = Nn // ISLAB
    NCH = Nn // ISLAB  # projection chunks per tensor
    DTP, DTA, DTO = cfg.dt_proj, cfg.dt_attn, cfg.dt_out

    NCH_ = Nn // min(512, Nn)
    xT = nc.dram_tensor(
        "xT", [NCH_, 128, KC, min(512, Nn)], DTP, kind="ExternalInput"
    ).ap()
    cT = nc.dram_tensor(
        "cT", [NCH_, 128, KC, min(512, Nn)], DTP, kind="ExternalInput"
    ).ap()
    wd = {
        name: nc.dram_tensor(name, [128, KC, 128], DTP, kind="ExternalInput").ap()
        for name in ("wq1", "wk1", "wv1", "wq2", "wk2", "wv2")
    }
    wout_d = nc.dram_tensor("wout", [128, D], DTO, kind="ExternalInput").ap()
    y_d = nc.dram_tensor("y", [Nn, D], F32, kind="ExternalOutput").ap()

    with tile.TileContext(nc) as tc:
        with (
            tc.tile_pool(name="const", bufs=1) as cpool,
            tc.tile_pool(name="qkv", bufs=1) as qkvpool,
            tc.tile_pool(name="vaug", bufs=1) as vaugpool,
            tc.tile_pool(name="outp", bufs=1) as outpool,
            tc.tile_pool(name="slab", bufs=8) as slabpool,
            tc.tile_pool(name="exp", bufs=6) as exppool,
            tc.tile_pool(name="tmp", bufs=4) as tmppool,
            tc.tile_pool(name="ysb", bufs=3) as ypool,
            tc.tile_pool(name="sim", bufs=2, space="PSUM") as simpool,
            tc.tile_pool(name="acc", bufs=2, space="PSUM") as accpool,
            tc.tile_pool(name="util", bufs=2, space="PSUM") as utilpool,
        ):
            # ---- input/weight DMAs, most-urgent first ----
            xs_tiles = {}
            chunk_done = set()
            SRCD = {"1": xT, "2": cT}

            def ensure_src_slab(stream, ch):
                if ("src", stream, ch) in chunk_done:
                    return
                chunk_done.add(("src", stream, ch))
                xs = slabpool.tile(
                    [128, KC, ISLAB], DTP, tag="xs", name=f"xs{stream}{ch}"
                )
                nc.sync.dma_start(out=xs[:], in_=SRCD[stream][ch])
                xs_tiles[(stream, ch)] = xs

            ensure_src_slab("2", 0)
            ensure_src_slab("1", 0)
            w_sb = {}
            for name in ("wq2", "wk1", "wq1", "wk2", "wv1", "wv2"):
                w_sb[name] = cpool.tile([128, KC, 128], DTP, tag=name, name=name)
                nc.sync.dma_start(out=w_sb[name][:], in_=wd[name])
            wout_sb = cpool.tile([128, D], DTO, tag="wout")
            nc.sync.dma_start(out=wout_sb[:], in_=wout_d)

            ones_f32 = cpool.tile([128, 64], F32, tag="ones_f32")
            nc.vector.memset(ones_f32[:], 1.0)
            ones_sb = cpool.tile([128, 64], F32R, tag="ones")
            nc.vector.tensor_copy(out=ones_sb[:], in_=ones_f32[:])
            ident_f32 = cpool.tile([128, 128], F32, tag="ident_f32")
            make_identity(nc, ident_f32[:])
            ident = cpool.tile([128, 128], DTA, tag="ident")
            nc.vector.tensor_copy(out=ident[:], in_=ident_f32[:])

            proj = {}
            for name in ("q1", "k1", "v1", "q2", "k2", "v2"):
                proj[name] = qkvpool.tile([128, Nn], DTA, tag=name, name=name)

            # vaug[br][j, jc, h, c]: transposed, alpha-scaled V for both
            # heads; c=64 is the shared softmax-denominator ones column.
            vaug = {}
            for br in (0, 1):
                vaug[br] = vaugpool.tile(
                    [128, NJC, 2, VHW], DTA, tag=f"vaug{br}", name=f"vaug{br}"
                )
                for h in (0, 1):
                    nc.vector.tensor_copy(
                        out=vaug[br][:, :, h, 64],
                        in_=ones_f32[:, 0:1].to_broadcast((128, NJC)),
                    )

            # outT rows 0..63 = head 0 (written directly); head 1 staged in
            # outTB then DMA-shifted into rows 64..127 (compute engines are
            # lane-aligned; DMA can move across partitions).
            outT = outpool.tile([128, Nn], DTO, tag="outT")
            outTB = outpool.tile([64, Nn], DTO, tag="outTB")

            # ---- streamed projection work (PE filler inside attention) ----
            def ensure_chunk(pname, ch):
                """Project chunk ch (ISLAB wide) of tensor pname (q/k)."""
                if (pname, ch) in chunk_done or ch >= NCH:
                    return
                chunk_done.add((pname, ch))
                stream = pname[1]
                ensure_src_slab(stream, ch)
                xs = xs_tiles[(stream, ch)]
                pp = utilpool.tile([128, ISLAB], F32, tag="util", name=f"pp{pname}{ch}")
                wt = w_sb["w" + pname]
                for kc in range(KC):
                    nc.tensor.matmul(
                        pp[:], wt[:, kc, :], xs[:, kc, :],
                        start=(kc == 0), stop=(kc == KC - 1),
                    )
                nc.vector.tensor_copy(
                    out=proj[pname][:, ch * ISLAB : (ch + 1) * ISLAB], in_=pp[:]
                )

            def ensure_vaug(br, ch):
                """Project + transpose chunk ch of v{br+1} into vaug."""
                if ("vaug", br, ch) in chunk_done or ch >= NCH:
                    return
                chunk_done.add(("vaug", br, ch))
                vname = "v1" if br == 0 else "v2"
                ensure_chunk(vname, ch)
                vt = proj[vname]
                for jc in range(ch * (ISLAB // 128), (ch + 1) * (ISLAB // 128)):
                    pt = utilpool.tile([128, ISLAB], DTA, tag="util", name=f"pt{br}{jc}")
                    nc.tensor.transpose(
                        pt[:, 0:128], vt[:, jc * 128 : (jc + 1) * 128], ident[:]
                    )
                    nc.vector.tensor_copy(
                        out=vaug[br][:, jc, :, 0:64],
                        in_=pt[:, 0:128].rearrange("p (h d) -> p h d", h=2),
                    )

            # Deferred PE work (output projection chunks, late q-projection
            # chunks, slab-normalize PE pieces) is queued and pumped a couple
            # of jobs per j-chunk so it fills PE slack without delaying the
            # score matmuls that feed ACT.
            jobs = []

            def pump(n=1):
                for _ in range(min(n, len(jobs))):
                    jobs.pop(0)()

            def queue_qchunk(pname, ch):
                """Front-queue one q-projection chunk as two half jobs."""
                if (pname, ch) in chunk_done or ch >= NCH:
                    return
                chunk_done.add((pname, ch))
                stream = pname[1]
                holder = {}

                def half1():
                    ensure_src_slab(stream, ch)
                    xs = xs_tiles[(stream, ch)]
                    pp = utilpool.tile(
                        [128, ISLAB], F32, tag="util", name=f"pp{pname}{ch}"
                    )
                    holder["pp"] = pp
                    wt = w_sb["w" + pname]
                    for kc in range(KC // 2):
                        nc.tensor.matmul(
                            pp[:], wt[:, kc, :], xs[:, kc, :],
                            start=(kc == 0), stop=False,
                        )

                def half2():
                    xs = xs_tiles[(stream, ch)]
                    pp = holder["pp"]
                    wt = w_sb["w" + pname]
                    for kc in range(KC // 2, KC):
                        nc.tensor.matmul(
                            pp[:], wt[:, kc, :], xs[:, kc, :],
                            start=False, stop=(kc == KC - 1),
                        )
                    nc.vector.tensor_copy(
                        out=proj[pname][:, ch * ISLAB : (ch + 1) * ISLAB],
                        in_=pp[:],
                    )

                jobs[:0] = [half1, half2]

            def queue_outproj(sl, last=False):
                ocw = min(512, D)
                nocs = D // ocw
                out = []
                for ic in range(sl * (ISLAB // 128), (sl + 1) * (ISLAB // 128)):
                    ysb = ypool.tile([128, D], F32, tag="ysb", name=f"ysb{ic}")

                    def job(ic=ic, ysb=ysb, oc=0):
                        icsl = slice(ic * 128, (ic + 1) * 128)
                        ocsl = slice(oc * ocw, (oc + 1) * ocw)
                        py = utilpool.tile(
                            [128, ISLAB], F32, tag="util", name=f"py{ic}{oc}"
                        )
                        nc.tensor.matmul(
                            py[:, 0:ocw], outT[:, icsl], wout_sb[:, ocsl],
                            start=True, stop=True,
                        )
                        if last:
                            nc.scalar.copy(out=ysb[:, ocsl], in_=py[:, 0:ocw])
                        else:
                            nc.vector.tensor_copy(
                                out=ysb[:, ocsl], in_=py[:, 0:ocw]
                            )
                        nc.sync.dma_start(
                            out=y_d[ic * 128 : (ic + 1) * 128, ocsl],
                            in_=ysb[:, ocsl],
                        )

                    for oc in range(nocs):
                        out.append(lambda ic=ic, ysb=ysb, oc=oc: job(ic, ysb, oc))
                return out

            # pair p=0: (br0, head0) + (br1, head1); p=1: (br0, head1) + (br1, head0)
            PAIRS = (((0, 0), (1, 1)), ((0, 1), (1, 0)))
            JPC = ISLAB // 128  # j-chunks per projection chunk

            def emit_scores(members, simP, sl, jc):
                i0 = sl * ISLAB
                jsl = slice(jc * 128, (jc + 1) * 128)
                for m, (br, h) in enumerate(members):
                    q = proj["q2"] if br == 0 else proj["q1"]
                    k = proj["k1"] if br == 0 else proj["k2"]
                    rs = slice(h * 64, h * 64 + 64)
                    nc.tensor.matmul(
                        simP[:, m, :], k[rs, jsl], q[rs, i0 : i0 + ISLAB],
                        start=True, stop=True, tile_position=(h * 64, 0),
                    )

            def emit_attnv(members, expP, accs, jc):
                for m, (br, h) in enumerate(members):
                    nc.tensor.matmul(
                        accs[(br, h)][0:65, :], vaug[br][:, jc, h, 0:65],
                        expP[:, m, :],
                        start=(jc == 0), stop=(jc == NJC - 1),
                    )

            bc_holder = {}

            def finish_slab(p, sl, accs, last=False):
                """Normalize both members of a finished slab.

                Emits the acc-PSUM reads (DVE only) inline so the acc slots
                rotate; defers the PE/mixed closures (broadcast matmul,
                mul/add, head-1 shift, output projection) into the job queue
                -- unless last, in which case everything is emitted now.
                """
                members = PAIRS[p]
                i0 = sl * ISLAB
                isl_ = slice(i0, i0 + ISLAB)
                deferred = []
                for br, h in members:
                    acc = accs[(br, h)]
                    ot = outT if h == 0 else outTB
                    # inline: drain acc (value copy + reciprocal) + f32r cast
                    if p == 0:
                        dst = ot[0:64, isl_]
                    else:
                        tmp = tmppool.tile(
                            [128, ISLAB], F32, tag="tmp", name=f"tmp{sl}{br}{h}"
                        )
                        dst = tmp[0:64, :]
                    nc.vector.tensor_copy(out=dst, in_=acc[0:64, :])
                    rcpf = tmppool.tile(
                        [128, ISLAB], F32, tag="rcpf", name=f"rcpf{sl}{br}{h}"
                    )
                    # the custom-DVE reciprocal requires base partition 0
                    nc.vector.reciprocal_approx_fast(
                        out=rcpf[0:65, :], in_=acc[0:65, :]
                    )
                    rcp = tmppool.tile(
                        [128, ISLAB], F32R, tag="rcpr", name=f"rcp{sl}{br}{h}"
                    )
                    nc.vector.tensor_copy(out=rcp[64:65, :], in_=rcpf[64:65, :])

                    def c_bc(br=br, h=h, rcp=rcp):
                        bc = utilpool.tile(
                            [128, ISLAB], F32, tag="util", name=f"bc{sl}{br}{h}"
                        )
                        nc.tensor.matmul(
                            bc[0:64, :], ones_sb[64:65, :], rcp[64:65, :],
                            start=True, stop=True,
                        )
                        bc_holder[(br, h)] = bc

                    def c_mul(br=br, h=h, ot=ot, dst=dst):
                        bc = bc_holder[(br, h)]
                        nc.vector.tensor_mul(out=dst, in0=dst, in1=bc[0:64, :])
                        if p != 0:
                            nc.vector.tensor_add(
                                out=ot[0:64, isl_], in0=ot[0:64, isl_], in1=dst
                            )

                    deferred.append(c_bc)
                    deferred.append(c_mul)
                if p == 1:
                    def c_shift():
                        nc.sync.dma_start(
                            out=outT[64:128, isl_], in_=outTB[0:64, isl_]
                        )

                    deferred.append(c_shift)
                    deferred.extend(queue_outproj(sl, last=last))
                if last:
                    for c in deferred:
                        c()
                else:
                    jobs[:0] = deferred

            # ---- first iteration, unrolled for fastest ACT spin-up ----
            members0 = PAIRS[0]
            ensure_chunk("q2", 0)
            ensure_chunk("k1", 0)
            accs = {
                (br, h): accpool.tile([128, ISLAB], F32, tag="acc", name=f"acc{br}{h}")
                for br, h in members0
            }
            simP = simpool.tile([128, 2, ISLAB], F32, tag="sim")
            expP = exppool.tile([128, 2, ISLAB], DTA, tag="exp")
            emit_scores(members0[:1], simP, 0, 0)
            nc.scalar.activation(
                expP[:, 0, :], simP[:, 0, :],
                mybir.ActivationFunctionType.Exp, scale=SCALE,
            )
            ensure_chunk("q1", 0)
            ensure_chunk("k2", 0)
            br, h = members0[1]
            q = proj["q2"] if br == 0 else proj["q1"]
            k = proj["k1"] if br == 0 else proj["k2"]
            rs = slice(h * 64, h * 64 + 64)
            nc.tensor.matmul(
                simP[:, 1, :], k[rs, 0:128], q[rs, 0:ISLAB],
                start=True, stop=True, tile_position=(h * 64, 0),
            )
            nc.scalar.activation(
                expP[:, 1, :], simP[:, 1, :],
                mybir.ActivationFunctionType.Exp, scale=SCALE,
            )
            ensure_vaug(0, 0)
            ensure_vaug(1, 0)
            emit_attnv(members0, expP, accs, 0)

            for p, members in enumerate(PAIRS):
                for sl in range(NSL):
                    first = p == 0 and sl == 0
                    if not first:
                        accs = {
                            (br, h): accpool.tile(
                                [128, ISLAB], F32, tag="acc", name=f"acc{br}{h}"
                            )
                            for br, h in members
                        }
                    for jc in range(1 if first else 0, NJC):
                        if first:
                            # paced k/v/vaug streaming, one piece per j-chunk;
                            # chunk c's four pieces land on jc = 4(c-1)+1 .. 4c,
                            # just before its own j-chunks begin at jc = 4c
                            nxt = (jc - 1) // JPC + 1
                            if nxt < NCH:
                                ph = (jc - 1) % JPC
                                if ph == 0:
                                    ensure_chunk("k2", nxt)
                                elif ph == 1:
                                    ensure_chunk("k1", nxt)
                                elif ph == 2:
                                    ensure_vaug(0, nxt)
                                else:
                                    ensure_vaug(1, nxt)
                        if p == 0 and jc in (8, 10) and sl + 1 < NSL:
                            queue_qchunk("q2" if jc == 8 else "q1", sl + 1)

                        simP = simpool.tile([128, 2, ISLAB], F32, tag="sim")
                        emit_scores(members, simP, sl, jc)
                        expP = exppool.tile([128, 2, ISLAB], DTA, tag="exp")
                        nc.scalar.activation(
                            expP[:], simP[:],
                            mybir.ActivationFunctionType.Exp, scale=SCALE,
                        )
                        # deferred work lands between exp and attn@V in the PE
                        # stream, where the PE is waiting on ACT anyway
                        pump(2)
                        emit_attnv(members, expP, accs, jc)

                    finish_slab(p, sl, accs, last=(p == 1 and sl == NSL - 1))
            pump(len(jobs))

    nc.compile()
    return nc


_CACHE = {}
_ACTIVE_CFG = Cfg()


def _get_nc():
    if "nc" not in _CACHE:
        _CACHE["nc"] = build_nc(_ACTIVE_CFG)
    return _CACHE["nc"]


def _tile_kpart(a, dt):
    """[K, M] -> [128, K//128, M] with element (p, kc, m) = a[kc*128+p, m]."""
    k, m = a.shape
    return np.ascontiguousarray(
        a.reshape(k // 128, 128, m).transpose(1, 0, 2)
    ).astype(_NP[dt])


def make_in_maps(x, context, Wq1, Wk1, Wv1, Wq2, Wk2, Wv2, alpha_attn, Wout, bout):
    cfg = _ACTIVE_CFG
    alpha = float(1.0 / (1.0 + np.exp(-np.float64(alpha_attn))))
    Wv1s = np.asarray(Wv1, np.float32) * np.float32(alpha)
    Wv2s = np.asarray(Wv2, np.float32) * np.float32(1.0 - alpha)

    def _chunked(a):
        t = _tile_kpart(a, cfg.dt_proj)  # [128, KC, N]
        w = min(512, cfg.N)
        return np.ascontiguousarray(
            t.reshape(128, cfg.KC, cfg.N // w, w).transpose(2, 0, 1, 3)
        )

    xT = [_chunked(np.asarray(x[b], np.float32).T) for b in range(B)]
    cT = [_chunked(np.asarray(context[b], np.float32).T) for b in range(B)]

    in_maps = []
    for c in range(N_CORES):
        b, hg = c // HG, c % HG
        cols = slice(hg * 128, (hg + 1) * 128)
        in_maps.append(
            {
                "xT": xT[b],
                "cT": cT[b],
                "wq1": _tile_kpart(np.asarray(Wq1, np.float32)[:, cols], cfg.dt_proj),
                "wk1": _tile_kpart(np.asarray(Wk1, np.float32)[:, cols], cfg.dt_proj),
                "wv1": _tile_kpart(Wv1s[:, cols], cfg.dt_proj),
                "wq2": _tile_kpart(np.asarray(Wq2, np.float32)[:, cols], cfg.dt_proj),
                "wk2": _tile_kpart(np.asarray(Wk2, np.float32)[:, cols], cfg.dt_proj),
                "wv2": _tile_kpart(Wv2s[:, cols], cfg.dt_proj),
                "wout": np.ascontiguousarray(
                    np.asarray(Wout, np.float32)[cols, :]
                ).astype(_NP[cfg.dt_out]),
            }
        )
    return in_maps


def run_device(in_maps, trace=False, tmpdir=None):
    nc = _get_nc()
    return bass_utils.run_bass_kernel_spmd(
        nc, in_maps, core_ids=list(range(N_CORES)), trace=trace, tmpdir=tmpdir
    )


def kernel(x, context, Wq1, Wk1, Wv1, Wq2, Wk2, Wv2, alpha_attn, Wout, bout):
    in_maps = make_in_maps(
        x, context, Wq1, Wk1, Wv1, Wq2, Wk2, Wv2, alpha_attn, Wout, bout
    )
    res = run_device(in_maps)
    bout32 = np.asarray(bout, np.float32)
    out = np.empty((B, N, QD), np.float32)
    for b in range(B):
        acc = res.results[b * HG]["y"].astype(np.float32).copy()
        for hg in range(1, HG):
            acc += res.results[b * HG + hg]["y"]
        out[b] = acc + bout32[None, :]
    return out


# revision 21
# speedup vs baseline: 1.2780x; 1.0372x over previous
"""BiCrossAttention Trainium2 kernel.

Shards the (B=2, H=8) problem across 8 NeuronCores as (batch, head-pair):
core c handles batch c//4 and heads {2*(c%4), 2*(c%4)+1}.  Each core
computes its two heads' QKV projections, both cross-attention branches,
and a partial output projection; the host sums the 4 per-batch partials
and adds the bias.

Device-side layout notes:
  - activations are passed pre-transposed/tiled: xT[p, kc, n] = x[n, kc*128+p]
  - matmuls run in bf16 (1 cyc/row); scores are computed transposed
    (simT[j, i]) so exp feeds the attn@V matmul directly as a stationary
    operand
  - the inner loop processes (branch0, head X) and (branch1, head Y)
    together: their K=64 score matmuls occupy disjoint PE row groups
    (partitions 0-63 vs 64-127) and run concurrently, and one Exp
    instruction covers both members' scores
  - V is projected wide (like q/k) then transposed per 128-j-chunk on the
    PE; a single strided cast drops the transposed [j, 2*64] block into
    vaug[br][j, jc, head, 0:64], with a softmax-denominator ones column
    at [.., head, 64] (so both heads' denominators land on acc row 64)
  - the softmax normalization is split: the acc-PSUM reads (value copy,
    reciprocal, f32r cast) are emitted inline at the slab boundary on DVE
    only, while the PE work (reciprocal broadcast matmul, mul/add, DMA
    head-shift, output projection) is deferred into the next slab's inner
    loop as pumped filler, keeping the PE stream dense so ACT (the exp
    engine, the true bottleneck at ~1.1us per iteration) never starves
  - QKV projection chunks are interleaved into the attention loop as PE
    filler; kv/vT streaming in the first slab pass is paced per j-chunk
  - alpha gating is folded into the V weights on the host
"""

import sys
import types

import numpy as np

for _p in ("/opt/trn_rl_repo",):
    if _p not in sys.path:
        sys.path.append(_p)

# Register the axon NTFF profile hook if the image's antenv lacks it (needed
# only when tracing; harmless otherwise).
try:
    import antenv

    if "antenv.axon_hooks" not in sys.modules:
        try:
            import antenv.axon_hooks  # noqa: F401
        except ImportError:
            _hooks = types.ModuleType("antenv.axon_hooks")
            _hook_holder = [None]
            _hooks.set_axon_ntff_profile_hook = lambda h: _hook_holder.__setitem__(0, h)
            _hooks.get_axon_ntff_profile_hook = lambda: _hook_holder[0]
            sys.modules["antenv.axon_hooks"] = _hooks
            antenv.axon_hooks = _hooks
            try:
                from trn_agent_boot.trn_boot import _ntff_profile_via_ctypes

                _hooks.set_axon_ntff_profile_hook(
                    _ntff_profile_via_ctypes("/opt/axon/libaxon_pjrt.so")
                )
            except Exception:
                pass
except Exception:
    pass

import ml_dtypes
import concourse.bacc as bacc
import concourse.mybir as mybir
import concourse.tile as tile
from concourse import bass_utils
from concourse.masks import make_identity

F32 = mybir.dt.float32
F32R = mybir.dt.float32r
BF16 = mybir.dt.bfloat16

_NP = {F32: np.float32, F32R: np.float32, BF16: ml_dtypes.bfloat16}

# Full problem constants
B, N, QD, CD, H, DH = 2, 2048, 1024, 1024, 8, 64
INNER = H * DH
SCALE = DH**-0.5
N_CORES = 8
HG = 4  # head-groups (of 2 heads) per batch

# vaug layout: [128 j, NJC, 2 heads, VHW] with [v (64) | ones | pad]
VHW = 80


class Cfg:
    def __init__(self, n=N, d=QD, dt_proj=BF16, dt_attn=BF16, dt_out=BF16):
        self.N = n          # sequence length
        self.D = d          # model dim (= QD = CD)
        self.KC = d // 128  # contraction chunks for projections
        self.ISLAB = min(512, n)   # attention i-slab / projection i-chunk
        self.NJC = n // 128  # j chunks (128 keys each)
        self.dt_proj = dt_proj
        self.dt_attn = dt_attn
        self.dt_out = dt_out


def build_nc(cfg: Cfg):
    """Builds the single-core program (SPMD across all 8 cores)."""
    nc = bacc.Bacc("TRN2", target_bir_lowering=False, debug=False)
    KC, Nn, D = cfg.KC, cfg.N, cfg.D
    ISLAB, NJC = cfg.ISLAB, cfg.NJC
    NSL = Nn // ISLAB
    NCH = Nn // ISLAB  # projection chunks per tensor
    DTP, DTA, DTO = cfg.dt_proj, cfg.dt_attn, cfg.dt_out

    NCH_ = Nn // min(512, Nn)
    xT = nc.dram_tensor(
        "xT", [NCH_, 128, KC, min(512, Nn)], DTP, kind="ExternalInput"
    ).ap()
    cT = nc.dram_tensor(
        "cT", [NCH_, 128, KC, min(512, Nn)], DTP, kind="ExternalInput"
    ).ap()
    wd = {
        name: nc.dram_tensor(name, [128, KC, 128], DTP, kind="ExternalInput").ap()
        for name in ("wq1", "wk1", "wv1", "wq2", "wk2", "wv2")
    }
    wout_d = nc.dram_tensor("wout", [128, D], DTO, kind="ExternalInput").ap()
    y_d = nc.dram_tensor("y", [Nn, D], F32, kind="ExternalOutput").ap()

    with tile.TileContext(nc) as tc:
        with (
            tc.tile_pool(name="const", bufs=1) as cpool,
            tc.tile_pool(name="qkv", bufs=1) as qkvpool,
            tc.tile_pool(name="vaug", bufs=1) as vaugpool,
            tc.tile_pool(name="outp", bufs=1) as outpool,
            tc.tile_pool(name="slab", bufs=8) as slabpool,
            tc.tile_pool(name="exp", bufs=6) as exppool,
            tc.tile_pool(name="tmp", bufs=4) as tmppool,
            tc.tile_pool(name="ysb", bufs=3) as ypool,
            tc.tile_pool(name="sim", bufs=2, space="PSUM") as simpool,
            tc.tile_pool(name="acc", bufs=2, space="PSUM") as accpool,
            tc.tile_pool(name="util", bufs=2, space="PSUM") as utilpool,
        ):
            # ---- input/weight DMAs, most-urgent first ----
            xs_tiles = {}
            chunk_done = set()
            SRCD = {"1": xT, "2": cT}

            def ensure_src_slab(stream, ch):
                if ("src", stream, ch) in chunk_done:
                    return
                chunk_done.add(("src", stream, ch))
                xs = slabpool.tile(
                    [128, KC, ISLAB], DTP, tag="xs", name=f"xs{stream}{ch}"
                )
                nc.sync.dma_start(out=xs[:], in_=SRCD[stream][ch])
                xs_tiles[(stream, ch)] = xs

            ensure_src_slab("2", 0)
            ensure_src_slab("1", 0)
            w_sb = {}
            for name in ("wq2", "wk1", "wq1", "wk2", "wv1", "wv2"):
                w_sb[name] = cpool.tile([128, KC, 128], DTP, tag=name, name=name)
                nc.sync.dma_start(out=w_sb[name][:], in_=wd[name])
            wout_sb = cpool.tile([128, D], DTO, tag="wout")
            nc.sync.dma_start(out=wout_sb[:], in_=wout_d)

            ones_f32 = cpool.tile([128, 64], F32, tag="ones_f32")
            nc.vector.memset(ones_f32[:], 1.0)
            ones_sb = cpool.tile([128, 64], F32R, tag="ones")
            nc.vector.tensor_copy(out=ones_sb[:], in_=ones_f32[:])
            ident_f32 = cpool.tile([128, 128], F32, tag="ident_f32")
            make_identity(nc, ident_f32[:])
            ident = cpool.tile([128, 128], DTA, tag="ident")
            nc.vector.tensor_copy(out=ident[:], in_=ident_f32[:])

            proj = {}
            for name in ("q1", "k1", "v1", "q2", "k2", "v2"):
                proj[name] = qkvpool.tile([128, Nn], DTA, tag=name, name=name)

            # vaug[br][j, jc, h, c]: transposed, alpha-scaled V for both
            # heads; c=64 is the shared softmax-denominator ones column.
            vaug = {}
            for br in (0, 1):
                vaug[br] = vaugpool.tile(
                    [128, NJC, 2, VHW], DTA, tag=f"vaug{br}", name=f"vaug{br}"
                )
                for h in (0, 1):
                    nc.vector.tensor_copy(
                        out=vaug[br][:, :, h, 64],
                        in_=ones_f32[:, 0:1].to_broadcast((128, NJC)),
                    )

            # outT rows 0..63 = head 0 (written directly); head 1 staged in
            # outTB then DMA-shifted into rows 64..127 (compute engines are
            # lane-aligned; DMA can move across partitions).
            outT = outpool.tile([128, Nn], DTO, tag="outT")
            outTB = outpool.tile([64, Nn], DTO, tag="outTB")

            # ---- streamed projection work (PE filler inside attention) ----
            def ensure_chunk(pname, ch):
                """Project chunk ch (ISLAB wide) of tensor pname (q/k)."""
                if (pname, ch) in chunk_done or ch >= NCH:
                    return
                chunk_done.add((pname, ch))
                stream = pname[1]
                ensure_src_slab(stream, ch)
                xs = xs_tiles[(stream, ch)]
                pp = utilpool.tile([128, ISLAB], F32, tag="util", name=f"pp{pname}{ch}")
                wt = w_sb["w" + pname]
                for kc in range(KC):
                    nc.tensor.matmul(
                        pp[:], wt[:, kc, :], xs[:, kc, :],
                        start=(kc == 0), stop=(kc == KC - 1),
                    )
                nc.vector.tensor_copy(
                    out=proj[pname][:, ch * ISLAB : (ch + 1) * ISLAB], in_=pp[:]
                )

            def ensure_vaug(br, ch):
                """Project + transpose chunk ch of v{br+1} into vaug."""
                if ("vaug", br, ch) in chunk_done or ch >= NCH:
                    return
                chunk_done.add(("vaug", br, ch))
                vname = "v1" if br == 0 else "v2"
                ensure_chunk(vname, ch)
                vt = proj[vname]
                for jc in range(ch * (ISLAB // 128), (ch + 1) * (ISLAB // 128)):
                    pt = utilpool.tile([128, ISLAB], DTA, tag="util", name=f"pt{br}{jc}")
                    nc.tensor.transpose(
                        pt[:, 0:128], vt[:, jc * 128 : (jc + 1) * 128], ident[:]
                    )
                    nc.vector.tensor_copy(
                        out=vaug[br][:, jc, :, 0:64],
                        in_=pt[:, 0:128].rearrange("p (h d) -> p h d", h=2),
                    )

            # Deferred PE work (output projection chunks, late q-projection
            # chunks, slab-normalize PE pieces) is queued and pumped a couple
            # of jobs per j-chunk so it fills PE slack without delaying the
            # score matmuls that feed ACT.
            jobs = []

            def pump(n=1):
                for _ in range(min(n, len(jobs))):
                    jobs.pop(0)()

            def queue_qchunk(pname, ch):
                """Front-queue one q-projection chunk as two half jobs."""
                if (pname, ch) in chunk_done or ch >= NCH:
                    return
                chunk_done.add((pname, ch))
                stream = pname[1]
                holder = {}

                def half1():
                    ensure_src_slab(stream, ch)
                    xs = xs_tiles[(stream, ch)]
                    pp = utilpool.tile(
                        [128, ISLAB], F32, tag="util", name=f"pp{pname}{ch}"
                    )
                    holder["pp"] = pp
                    wt = w_sb["w" + pname]
                    for kc in range(KC // 2):
                        nc.tensor.matmul(
                            pp[:], wt[:, kc, :], xs[:, kc, :],
                            start=(kc == 0), stop=False,
                        )

                def half2():
                    xs = xs_tiles[(stream, ch)]
                    pp = holder["pp"]
                    wt = w_sb["w" + pname]
                    for kc in range(KC // 2, KC):
                        nc.tensor.matmul(
                            pp[:], wt[:, kc, :], xs[:, kc, :],
                            start=False, stop=(kc == KC - 1),
                        )
                    nc.vector.tensor_copy(
                        out=proj[pname][:, ch * ISLAB : (ch + 1) * ISLAB],
                        in_=pp[:],
                    )

                jobs[:0] = [half1, half2]

            def queue_outproj(sl, last=False):
                ocw = min(512, D)
                nocs = D // ocw
                out = []
                for ic in range(sl * (ISLAB // 128), (sl + 1) * (ISLAB // 128)):
                    ysb = ypool.tile([128, D], F32, tag="ysb", name=f"ysb{ic}")

                    def job(ic=ic, ysb=ysb, oc=0):
                        icsl = slice(ic * 128, (ic + 1) * 128)
                        ocsl = slice(oc * ocw, (oc + 1) * ocw)
                        py = utilpool.tile(
                            [128, ISLAB], F32, tag="util", name=f"py{ic}{oc}"
                        )
                        nc.tensor.matmul(
                            py[:, 0:ocw], outT[:, icsl], wout_sb[:, ocsl],
                            start=True, stop=True,
                        )
                        if last:
                            nc.scalar.copy(out=ysb[:, ocsl], in_=py[:, 0:ocw])
                        else:
                            nc.vector.tensor_copy(
                                out=ysb[:, ocsl], in_=py[:, 0:ocw]
                            )
                        nc.sync.dma_start(
                            out=y_d[ic * 128 : (ic + 1) * 128, ocsl],
                            in_=ysb[:, ocsl],
                        )

                    for oc in range(nocs):
                        out.append(lambda ic=ic, ysb=ysb, oc=oc: job(ic, ysb, oc))
                return out

            # pair p=0: (br0, head0) + (br1, head1); p=1: (br0, head1) + (br1, head0)
            PAIRS = (((0, 0), (1, 1)), ((0, 1), (1, 0)))
            JPC = ISLAB // 128  # j-chunks per projection chunk

            def emit_scores(members, simP, sl, jc):
                i0 = sl * ISLAB
                jsl = slice(jc * 128, (jc + 1) * 128)
                for m, (br, h) in enumerate(members):
                    q = proj["q2"] if br == 0 else proj["q1"]
                    k = proj["k1"] if br == 0 else proj["k2"]
                    rs = slice(h * 64, h * 64 + 64)
                    nc.tensor.matmul(
                        simP[:, m, :], k[rs, jsl], q[rs, i0 : i0 + ISLAB],
                        start=True, stop=True, tile_position=(h * 64, 0),
                    )

            def emit_attnv(members, expP, accs, jc):
                for m, (br, h) in enumerate(members):
                    nc.tensor.matmul(
                        accs[(br, h)][0:65, :], vaug[br][:, jc, h, 0:65],
                        expP[:, m, :],
                        start=(jc == 0), stop=(jc == NJC - 1),
                    )

            bc_holder = {}

            def finish_slab(p, sl, accs, last=False):
                """Normalize both members of a finished slab.

                Emits the acc-PSUM reads (DVE only) inline so the acc slots
                rotate; defers the PE/mixed closures (broadcast matmul,
                mul/add, head-1 shift, output projection) into the job queue
                -- unless last, in which case everything is emitted now.
                """
                members = PAIRS[p]
                i0 = sl * ISLAB
                isl_ = slice(i0, i0 + ISLAB)
                deferred = []
                for br, h in members:
                    acc = accs[(br, h)]
                    ot = outT if h == 0 else outTB
                    # inline: drain acc (value copy + reciprocal) + f32r cast
                    if p == 0:
                        dst = ot[0:64, isl_]
                    else:
                        tmp = tmppool.tile(
                            [128, ISLAB], F32, tag="tmp", name=f"tmp{sl}{br}{h}"
                        )
                        dst = tmp[0:64, :]
                    nc.vector.tensor_copy(out=dst, in_=acc[0:64, :])
                    rcpf = tmppool.tile(
                        [128, ISLAB], F32, tag="rcpf", name=f"rcpf{sl}{br}{h}"
                    )
                    # the custom-DVE reciprocal requires base partition 0
                    nc.vector.reciprocal_approx_fast(
                        out=rcpf[0:65, :], in_=acc[0:65, :]
                    )
                    rcp = tmppool.tile(
                        [128, ISLAB], F32R, tag="rcpr", name=f"rcp{sl}{br}{h}"
                    )
                    nc.vector.tensor_copy(out=rcp[64:65, :], in_=rcpf[64:65, :])

                    def c_bc(br=br, h=h, rcp=rcp):
                        bc = utilpool.tile(
                            [128, ISLAB], F32, tag="util", name=f"bc{sl}{br}{h}"
                        )
                        nc.tensor.matmul(
                            bc[0:64, :], ones_sb[64:65, :], rcp[64:65, :],
                            start=True, stop=True,
                        )
                        bc_holder[(br, h)] = bc

                    def c_mul(br=br, h=h, ot=ot, dst=dst):
                        bc = bc_holder[(br, h)]
                        nc.vector.tensor_mul(out=dst, in0=dst, in1=bc[0:64, :])
                        if p != 0:
                            nc.vector.tensor_add(
                                out=ot[0:64, isl_], in0=ot[0:64, isl_], in1=dst
                            )

                    deferred.append(c_bc)
                    deferred.append(c_mul)
                if p == 1:
                    def c_shift():
                        nc.sync.dma_start(
                            out=outT[64:128, isl_], in_=outTB[0:64, isl_]
                        )

                    deferred.append(c_shift)
                    deferred.extend(queue_outproj(sl, last=last))
                if last:
                    for c in deferred:
                        c()
                else:
                    jobs[:0] = deferred

            # ---- first iteration, unrolled for fastest ACT spin-up ----
            members0 = PAIRS[0]
            ensure_chunk("q2", 0)
            ensure_chunk("k1", 0)
            accs = {
                (br, h): accpool.tile([128, ISLAB], F32, tag="acc", name=f"acc{br}{h}")
                for br, h in members0
            }
            simP = simpool.tile([128, 2, ISLAB], F32, tag="sim")
            expP = exppool.tile([128, 2, ISLAB], DTA, tag="exp")
            emit_scores(members0[:1], simP, 0, 0)
            nc.scalar.activation(
                expP[:, 0, :], simP[:, 0, :],
                mybir.ActivationFunctionType.Exp, scale=SCALE,
            )
            ensure_chunk("q1", 0)
            ensure_chunk("k2", 0)
            br, h = members0[1]
            q = proj["q2"] if br == 0 else proj["q1"]
            k = proj["k1"] if br == 0 else proj["k2"]
            rs = slice(h * 64, h * 64 + 64)
            nc.tensor.matmul(
                simP[:, 1, :], k[rs, 0:128], q[rs, 0:ISLAB],
                start=True, stop=True, tile_position=(h * 64, 0),
            )
            nc.scalar.activation(
                expP[:, 1, :], simP[:, 1, :],
                mybir.ActivationFunctionType.Exp, scale=SCALE,
            )
            ensure_vaug(0, 0)
            ensure_vaug(1, 0)
            pend = (expP, 0)  # software-pipeline skew: attnV runs 1 jc behind

            for p, members in enumerate(PAIRS):
                for sl in range(NSL):
                    first = p == 0 and sl == 0
                    if not first:
                        accs = {
                            (br, h): accpool.tile(
                                [128, ISLAB], F32, tag="acc", name=f"acc{br}{h}"
                            )
                            for br, h in members
                        }
                    for jc in range(1 if first else 0, NJC):
                        if first:
                            # paced k/v/vaug streaming, one piece per j-chunk;
                            # chunk c's four pieces land on jc = 4(c-1)+1 .. 4c,
                            # just before its own j-chunks begin at jc = 4c
                            nxt = (jc - 1) // JPC + 1
                            if nxt < NCH:
                                ph = (jc - 1) % JPC
                                if ph == 0:
                                    ensure_chunk("k2", nxt)
                                elif ph == 1:
                                    ensure_chunk("k1", nxt)
                                elif ph == 2:
                                    ensure_vaug(0, nxt)
                                else:
                                    ensure_vaug(1, nxt)
                        if p == 0 and jc in (8, 10) and sl + 1 < NSL:
                            queue_qchunk("q2" if jc == 8 else "q1", sl + 1)

                        simP = simpool.tile([128, 2, ISLAB], F32, tag="sim")
                        emit_scores(members, simP, sl, jc)
                        expP = exppool.tile([128, 2, ISLAB], DTA, tag="exp")
                        nc.scalar.activation(
                            expP[:], simP[:],
                            mybir.ActivationFunctionType.Exp, scale=SCALE,
                        )
                        # deferred work lands between exp and the (skewed)
                        # attn@V in the PE stream, where PE waits on ACT anyway
                        pump(2)
                        if pend is not None:
                            emit_attnv(members, pend[0], accs, pend[1])
                        pend = (expP, jc)

                    # drain the skew before normalizing this slab
                    emit_attnv(members, pend[0], accs, pend[1])
                    pend = None
                    finish_slab(p, sl, accs, last=(p == 1 and sl == NSL - 1))
            pump(len(jobs))

    nc.compile()
    return nc


_CACHE = {}
_ACTIVE_CFG = Cfg()


def _get_nc():
    if "nc" not in _CACHE:
        _CACHE["nc"] = build_nc(_ACTIVE_CFG)
    return _CACHE["nc"]


def _tile_kpart(a, dt):
    """[K, M] -> [128, K//128, M] with element (p, kc, m) = a[kc*128+p, m]."""
    k, m = a.shape
    return np.ascontiguousarray(
        a.reshape(k // 128, 128, m).transpose(1, 0, 2)
    ).astype(_NP[dt])


def make_in_maps(x, context, Wq1, Wk1, Wv1, Wq2, Wk2, Wv2, alpha_attn, Wout, bout):
    cfg = _ACTIVE_CFG
    alpha = float(1.0 / (1.0 + np.exp(-np.float64(alpha_attn))))
    Wv1s = np.asarray(Wv1, np.float32) * np.float32(alpha)
    Wv2s = np.asarray(Wv2, np.float32) * np.float32(1.0 - alpha)

    def _chunked(a):
        t = _tile_kpart(a, cfg.dt_proj)  # [128, KC, N]
        w = min(512, cfg.N)
        return np.ascontiguousarray(
            t.reshape(128, cfg.KC, cfg.N // w, w).transpose(2, 0, 1, 3)
        )

    xT = [_chunked(np.asarray(x[b], np.float32).T) for b in range(B)]
    cT = [_chunked(np.asarray(context[b], np.float32).T) for b in range(B)]

    in_maps = []
    for c in range(N_CORES):
        b, hg = c // HG, c % HG
        cols = slice(hg * 128, (hg + 1) * 128)
        in_maps.append(
            {
                "xT": xT[b],
                "cT": cT[b],
                "wq1": _tile_kpart(np.asarray(Wq1, np.float32)[:, cols], cfg.dt_proj),
                "wk1": _tile_kpart(np.asarray(Wk1, np.float32)[:, cols], cfg.dt_proj),
                "wv1": _tile_kpart(Wv1s[:, cols], cfg.dt_proj),
                "wq2": _tile_kpart(np.asarray(Wq2, np.float32)[:, cols], cfg.dt_proj),
                "wk2": _tile_kpart(np.asarray(Wk2, np.float32)[:, cols], cfg.dt_proj),
                "wv2": _tile_kpart(Wv2s[:, cols], cfg.dt_proj),
                "wout": np.ascontiguousarray(
                    np.asarray(Wout, np.float32)[cols, :]
                ).astype(_NP[cfg.dt_out]),
            }
        )
    return in_maps


def run_device(in_maps, trace=False, tmpdir=None):
    nc = _get_nc()
    return bass_utils.run_bass_kernel_spmd(
        nc, in_maps, core_ids=list(range(N_CORES)), trace=trace, tmpdir=tmpdir
    )


def kernel(x, context, Wq1, Wk1, Wv1, Wq2, Wk2, Wv2, alpha_attn, Wout, bout):
    in_maps = make_in_maps(
        x, context, Wq1, Wk1, Wv1, Wq2, Wk2, Wv2, alpha_attn, Wout, bout
    )
    res = run_device(in_maps)
    bout32 = np.asarray(bout, np.float32)
    out = np.empty((B, N, QD), np.float32)
    for b in range(B):
        acc = res.results[b * HG]["y"].astype(np.float32).copy()
        for hg in range(1, HG):
            acc += res.results[b * HG + hg]["y"]
        out[b] = acc + bout32[None, :]
    return out
